# revision 20
# baseline (speedup 1.0000x reference)
"""Trainium2 Bass kernel for LFGA-style attention block (raw Bass, 8-core SPMD).

Per-batch (B=8, C=256, H=W=64, N=4096, CQ=64), one batch element per core:
    q/k = Wq/Wk @ fb + b   [64, N];  v = (0.5*Wv) @ (2*fa) + bv  [C, N]
    S2[j,i] = k.q (energy TRANSPOSED so softmax dim j is on partitions)
    A2 = exp(S2 + bias);  O_un[c,i] = sum_j vT[j,c] A2[j,i]
    s[i] = sum_j A2[j,i] (DVE chunk-accumulate + ones-matmul partition reduce)
    y' = (16*gamma/s) * O_un   -> fp8-e3m4 DRAM
Host: out = relu(y'/16 + fa).

Wire-format optimization (axon-tunneled cores; transfers dominate wall time):
    fa ships as fp8-e3m4 scaled x2 (denormal-floor fix; 0.5 folded into Wv),
    fb/weights ship bf16, y ships fp8-e3m4 scaled x16 (fold into gamma).
    Simulated end-to-end rel l2 err 1.6e-3 vs the 2e-2 gate.
Dispatch: cached jit(shard_map(bass_exec)) without donated zero output
    buffers (kernel writes every output element), avoiding 34MB of
    zero-buffer H2D per call. Falls back to run_bass_kernel_spmd.
"""

import numpy as np
import ml_dtypes

import concourse.bass as bass
import concourse.mybir as mybir

P = 128
B, C, HW = 8, 256, 64
N = HW * HW
CQ = 64
NT = 512
NIT = N // NT        # 8
NJ = N // P          # 32
F32 = mybir.dt.float32
BF16 = mybir.dt.bfloat16
E3 = mybir.dt.float8e3
NP_E3 = ml_dtypes.float8_e3m4
NP_BF16 = ml_dtypes.bfloat16
EXP_BIAS = -20.0
FA_SCALE = 2.0       # fa quantization pre-scale (1/2 folded into Wv)
Y_SCALE = 16.0       # output quantization pre-scale (folded into gamma)
AF = mybir.ActivationFunctionType

# engine stream bases / sizes
# input loads are split into two gates: qsem (fb/wq/wk/bqk - what the q/k
# phase needs) and dsem (fa/wv/bv/gamma - first needed by the vT phase),
# so the fa/wv DMAs overlap the q/k matmuls instead of heading the kernel.
QS0 = 4 * 16                 # qsem after early input loads
DS0 = 4 * 16                 # dsem after late input loads
TQKV = 32 + 96               # PE matmuls in qkv phase
PEIT = 98                    # PE matmuls per i-tile
AQKV = 16 + 32               # ACT ops in qkv phase
AIT = 35                     # ACT ops per i-tile
VS0 = 3                      # DVE memsets
VIT = 36                     # DVE ops per i-tile

_CACHE = {}


def _pos_s2(jj):
    return jj + 1 if jj < 4 else 3 * jj - 7


def _pos_oc1(jb):
    return 3 * jb + 7 if jb <= 27 else 2 * jb + 34


def _build():
    nc = bass.Bass()

    # packed inputs: per-call dispatch cost scales with parameter count
    # (~50us/param/call over axon), so ship 2 params instead of 5.
    # pk row c (bf16 units): [fa_c as e3m4 bytes: 0:N/2 | fb_c: N/2:N/2+N
    #                         | wq_c|wk_c|wv_c: N/2+N : N/2+N+2CQ+C]
    PKW = N // 2 + N + 2 * CQ + C
    pk = nc.declare_dram_parameter("pk", [C, PKW], BF16, isOutput=False)
    bqkd = nc.declare_dram_parameter("bqk", [2 * CQ, 1], F32, isOutput=False)
    browd = nc.declare_dram_parameter("brow", [1, C + 1], F32, isOutput=False)
    out = nc.declare_dram_parameter("out", [C, N], E3, isOutput=True)

    pk3e = pk.bitcast(E3).rearrange("(o p) m -> p o m", p=P)
    pk3 = pk.rearrange("(o p) m -> p o m", p=P)
    fa3 = pk3e[:, :, 0:N]
    fb3 = pk3[:, :, N // 2:N // 2 + N]
    wp3 = pk3[:, :, N // 2 + N:]
    out3 = out.rearrange("(o p) n -> p o n", p=P)

    def T0(it):
        return TQKV + PEIT * it

    def A0(it):
        return AQKV + AIT * it

    def V0(it):
        return VS0 + VIT * it

    from contextlib import ExitStack
    with ExitStack() as _es:
        fa_sb = _es.enter_context(nc.sbuf_tensor([P, 2, N], E3))
        fb_sb = _es.enter_context(nc.sbuf_tensor([P, 2, N], BF16))
        wq_sb = _es.enter_context(nc.sbuf_tensor([P, 2, CQ], BF16))
        wk_sb = _es.enter_context(nc.sbuf_tensor([P, 2, CQ], BF16))
        wv_sb = _es.enter_context(nc.sbuf_tensor([P, 2, C], BF16))
        bqk_sb = _es.enter_context(nc.sbuf_tensor([2 * CQ, 1], F32))
        bv_sb = _es.enter_context(nc.sbuf_tensor([1, C], F32))
        gam_sb = _es.enter_context(nc.sbuf_tensor([1, 1], F32))
        onesc = _es.enter_context(nc.sbuf_tensor([P, 1], F32))
        onesr = _es.enter_context(nc.sbuf_tensor([1, P], F32))
        expb = _es.enter_context(nc.sbuf_tensor([P, 1], F32))
        q_sb = _es.enter_context(nc.sbuf_tensor([CQ, N], BF16))
        k_sb = _es.enter_context(nc.sbuf_tensor([CQ, N], BF16))
        vT_sb = _es.enter_context(nc.sbuf_tensor([P, NJ, C], BF16))
        a2_sb = _es.enter_context(nc.sbuf_tensor([P, 4, NT], BF16))
        acc_sb = _es.enter_context(nc.sbuf_tensor([P, 2, NT], F32))
        r_sb = _es.enter_context(nc.sbuf_tensor([1, 2, NT], F32))
        rb_sb = _es.enter_context(nc.sbuf_tensor([P, NT], F32))
        t1_sb = _es.enter_context(nc.sbuf_tensor([P, 2, NT], F32))
        ot0_sb = _es.enter_context(nc.sbuf_tensor([P, 2, NT], E3))
        ot1_sb = _es.enter_context(nc.sbuf_tensor([P, 2, NT], E3))
        pp0 = _es.enter_context(nc.psum_tensor([P, NT], F32))
        pp1 = _es.enter_context(nc.psum_tensor([P, NT], F32))
        s2a = _es.enter_context(nc.psum_tensor([P, NT], F32))
        s2b = _es.enter_context(nc.psum_tensor([P, NT], F32))
        oc0p = _es.enter_context(nc.psum_tensor([P, NT], F32))
        oc1p = _es.enter_context(nc.psum_tensor([P, NT], F32))
        srow = _es.enter_context(nc.psum_tensor([1, NT], F32))
        rbp = _es.enter_context(nc.psum_tensor([P, NT], F32))
        dsem = _es.enter_context(nc.semaphore())
        qsem = _es.enter_context(nc.semaphore())
        tsem = _es.enter_context(nc.semaphore())
        asem = _es.enter_context(nc.semaphore())
        vsem = _es.enter_context(nc.semaphore())
        block = _es.enter_context(nc.Block())
        pp = [pp0, pp1]
        s2p = [s2a, s2b, pp0, pp1]
        ocp = [oc0p, oc1p]

        @block.sync
        def _(sync):
            for dst, src in ((fb_sb[:], fb3),
                             (wq_sb[:], wp3[:, :, 0:CQ]),
                             (wk_sb[:], wp3[:, :, CQ:2 * CQ]),
                             (bqk_sb[:], bqkd[:])):
                sync.dma_start(dst, src).then_inc(qsem, 16)
            for dst, src in ((fa_sb[:], fa3),
                             (wv_sb[:], wp3[:, :, 2 * CQ:]),
                             (bv_sb[:], browd[:, 0:C]),
                             (gam_sb[:], browd[:, C:C + 1])):
                sync.dma_start(dst, src).then_inc(dsem, 16)
            for it in range(NIT):
                isl = slice(it * NT, (it + 1) * NT)
                for cc, ot in ((0, ot0_sb), (1, ot1_sb)):
                    sync.wait_ge(asem, A0(it) + 34 + cc)
                    sync.dma_start(out3[:, cc, isl], ot[:, it % 2]).then_inc(dsem, 16)

        @block.tensor
        def _(tensor):
            tensor.wait_ge(qsem, QS0)
            tensor.wait_ge(vsem, VS0)
            # q, k tiles (n = 2t -> q, 2t+1 -> k)
            for n in range(16):
                t = n // 2
                sl = slice(t * NT, (t + 1) * NT)
                w = wq_sb if n % 2 == 0 else wk_sb
                if n >= 2:
                    tensor.wait_ge(asem, n - 1)
                pq = pp[n % 2][0:CQ]
                nc.tensor.matmul(pq, lhsT=w[:, 0], rhs=fb_sb[:, 0, sl],
                                 start=True, stop=False).then_inc(tsem, 1)
                nc.tensor.matmul(pq, lhsT=w[:, 1], rhs=fb_sb[:, 1, sl],
                                 start=False, stop=True).then_inc(tsem, 1)
            # vT tiles
            tensor.wait_ge(dsem, DS0)
            for n in range(NJ):
                jsl = slice(n * P, (n + 1) * P)
                tensor.wait_ge(asem, 16 + max(0, n - 1))
                pv = pp[n % 2][:, 0:C]
                nc.tensor.matmul(pv, lhsT=fa_sb[:, 0, jsl], rhs=wv_sb[:, 0],
                                 start=True, stop=False).then_inc(tsem, 1)
                nc.tensor.matmul(pv, lhsT=fa_sb[:, 1, jsl], rhs=wv_sb[:, 1],
                                 start=False, stop=False).then_inc(tsem, 1)
                nc.tensor.matmul(pv, lhsT=onesr[:], rhs=bv_sb[:],
                                 start=False, stop=True).then_inc(tsem, 1)
            # main loop
            for it in range(NIT):
                isl = slice(it * NT, (it + 1) * NT)

                def s2_mm(jj, it=it, isl=isl):
                    if jj < 4:
                        # bank s2p[jj%4] last read by exp(it-1, 28+jj); for
                        # it=0 the pp banks are read through the qkv phase
                        tensor.wait_ge(asem, AQKV if it == 0
                                       else A0(it) + jj - 6)
                    else:
                        tensor.wait_ge(asem, A0(it) + jj - 3)
                    jsl = slice(jj * P, (jj + 1) * P)
                    nc.tensor.matmul(s2p[jj % 4][:], lhsT=k_sb[:, jsl],
                                     rhs=q_sb[:, isl],
                                     start=True, stop=True).then_inc(tsem, 1)

                for jj in range(4):
                    s2_mm(jj)
                for jb in range(NJ):
                    if jb + 4 < NJ:
                        s2_mm(jb + 4)
                    tensor.wait_ge(asem, A0(it) + jb + 1)
                    if jb == 0 and it > 0:
                        tensor.wait_ge(vsem, V0(it))
                    nc.tensor.matmul(ocp[0][:], lhsT=vT_sb[:, jb, 0:P],
                                     rhs=a2_sb[:, jb % 4],
                                     start=(jb == 0), stop=(jb == NJ - 1)
                                     ).then_inc(tsem, 1)
                    nc.tensor.matmul(ocp[1][:], lhsT=vT_sb[:, jb, P:C],
                                     rhs=a2_sb[:, jb % 4],
                                     start=(jb == 0), stop=(jb == NJ - 1)
                                     ).then_inc(tsem, 1)
                tensor.wait_ge(vsem, V0(it) + 32)
                nc.tensor.matmul(srow[:], lhsT=onesc[:], rhs=acc_sb[:, it % 2],
                                 start=True, stop=True).then_inc(tsem, 1)
                tensor.wait_ge(vsem, V0(it) + 34)
                nc.tensor.matmul(rbp[:], lhsT=onesr[:], rhs=r_sb[:, it % 2],
                                 start=True, stop=True).then_inc(tsem, 1)

        @block.scalar
        def _(scalar):
            # q/k bias-add moves
            for n in range(16):
                t = n // 2
                sl = slice(t * NT, (t + 1) * NT)
                scalar.wait_ge(tsem, 2 * (n + 1))
                dst = q_sb if n % 2 == 0 else k_sb
                bias = bqk_sb[0:CQ] if n % 2 == 0 else bqk_sb[CQ:2 * CQ]
                nc.scalar.activation(dst[:, sl], pp[n % 2][0:CQ], AF.Identity,
                                     bias=bias).then_inc(asem, 1)
            # vT copies
            for n in range(NJ):
                scalar.wait_ge(tsem, 32 + 3 * (n + 1))
                nc.scalar.copy(vT_sb[:, n], pp[n % 2][:, 0:C]).then_inc(asem, 1)
            # main loop
            for it in range(NIT):
                for jb in range(NJ):
                    scalar.wait_ge(tsem, T0(it) + _pos_s2(jb))
                    if jb >= 4:
                        scalar.wait_ge(tsem, T0(it) + _pos_oc1(jb - 4))
                        scalar.wait_ge(vsem, V0(it) + jb - 3)
                    elif it > 0:
                        scalar.wait_ge(tsem, T0(it - 1) + _pos_oc1(jb + 28))
                        scalar.wait_ge(vsem, V0(it - 1) + jb + 29)
                    nc.scalar.activation(a2_sb[:, jb % 4], s2p[jb % 4][:], AF.Exp,
                                         bias=expb[:]).then_inc(asem, 1)
                scalar.wait_ge(tsem, T0(it) + 98)
                if it > 0:
                    scalar.wait_ge(vsem, V0(it))
                nc.scalar.copy(rb_sb[:], rbp[:]).then_inc(asem, 1)
                for cc, ot in ((0, ot0_sb), (1, ot1_sb)):
                    scalar.wait_ge(vsem, V0(it) + 35 + cc)
                    if it >= 2:
                        scalar.wait_ge(dsem, DS0 + 16 * 2 * (it - 1))
                    nc.scalar.copy(ot[:, it % 2], t1_sb[:, cc]).then_inc(asem, 1)

        @block.vector
        def _(vector):
            nc.vector.memset(onesc[:], 1.0).then_inc(vsem, 1)
            nc.vector.memset(onesr[:], 1.0).then_inc(vsem, 1)
            nc.vector.memset(expb[:], EXP_BIAS).then_inc(vsem, 1)
            vector.wait_ge(dsem, DS0)
            for it in range(NIT):
                for jb in range(NJ):
                    vector.wait_ge(asem, A0(it) + jb + 1)
                    if jb == 0:
                        if it >= 2:
                            vector.wait_ge(tsem, T0(it - 2) + 97)
                        nc.vector.tensor_copy(out=acc_sb[:, it % 2],
                                              in_=a2_sb[:, jb % 4]
                                              ).then_inc(vsem, 1)
                    else:
                        nc.vector.tensor_add(out=acc_sb[:, it % 2],
                                             in0=acc_sb[:, it % 2],
                                             in1=a2_sb[:, jb % 4]
                                             ).then_inc(vsem, 1)
                vector.wait_ge(tsem, T0(it) + 97)
                nc.vector.reciprocal(r_sb[:, it % 2], srow[:]).then_inc(vsem, 1)
                nc.vector.tensor_scalar_mul(r_sb[:, it % 2], r_sb[:, it % 2],
                                            gam_sb[:]).then_inc(vsem, 1)
                vector.wait_ge(tsem, T0(it) + 96)
                vector.wait_ge(asem, A0(it) + 33)
                for cc in (0, 1):
                    nc.vector.tensor_mul(out=t1_sb[:, cc], in0=ocp[cc][:],
                                         in1=rb_sb[:]).then_inc(vsem, 1)

    return nc


def _get_nc():
    if "nc" not in _CACHE:
        _CACHE["nc"] = _build()
    return _CACHE["nc"]


def _get_dispatch():
    """Cached jit(shard_map(bass_exec)) without donated zero output buffers."""
    if "fn" in _CACHE:
        return _CACHE["fn"], _CACHE["in_names"], _CACHE["out_names"]
    import jax
    from jax.sharding import Mesh, PartitionSpec
    from jax.experimental.shard_map import shard_map
    from concourse import bass2jax

    nc = _get_nc()
    bass2jax.install_neuronx_cc_hook()
    pname = nc.partition_id_tensor.name if nc.partition_id_tensor else None
    in_names, out_names, out_avals = [], [], []
    for alloc in nc.m.functions[0].allocations:
        if not isinstance(alloc, mybir.MemoryLocationSet):
            continue
        name = alloc.memorylocations[0].name
        if alloc.kind == "ExternalInput":
            if name != pname:
                in_names.append(name)
        elif alloc.kind == "ExternalOutput":
            out_names.append(name)
            out_avals.append(jax.core.ShapedArray(
                tuple(alloc.tensor_shape), mybir.dt.np(alloc.dtype)))
    n_params = len(in_names)
    bind_names = list(in_names) + ([pname] if pname is not None else [])

    def _body(*args):
        operands = list(args)
        if pname is not None:
            operands.append(bass2jax.partition_id_tensor())
        return tuple(bass2jax._bass_exec_p.bind(
            *operands,
            out_avals=tuple(out_avals),
            in_names=tuple(bind_names),
            out_names=tuple(out_names),
            lowering_input_output_aliases=(),
            sim_require_finite=True,
            sim_require_nnan=True,
            nc=nc,
        ))

    devices = jax.devices()[:B]
    mesh = Mesh(np.asarray(devices), ("core",))
    fn = jax.jit(shard_map(
        _body, mesh=mesh,
        in_specs=(PartitionSpec("core"),) * n_params,
        out_specs=(PartitionSpec("core"),) * len(out_names),
        check_rep=False))
    _CACHE["fn"] = fn
    _CACHE["in_names"] = in_names
    _CACHE["out_names"] = out_names
    return fn, in_names, out_names


def _quant_fa(fa):
    """fa -> e3m4 wire format: pre-scale x2, clip inside e3m4 range (+-15.5)
    so out-of-range values saturate instead of becoming inf."""
    x = fa * FA_SCALE
    np.clip(x, -15.5, 15.5, out=x)
    return x.astype(NP_E3)


def _get_sharding():
    if "sh" not in _CACHE:
        import jax
        from jax.sharding import Mesh, PartitionSpec, NamedSharding
        mesh = Mesh(np.asarray(jax.devices()[:B]), ("core",))
        _CACHE["sh"] = NamedSharding(mesh, PartitionSpec("core"))
    return _CACHE["sh"]


def _prep_inputs(inputs):
    """Full inputs -> packed wire arrays {pk, bias}, batch-major axis 0.

    pk row c (bytes): fa_c e3m4 [0:N) | fb_c bf16 [N:3N) | w_c bf16 [3N:3N+768)
    bias: [129, 257] f32 per core: rows 0:128 col 0 = bq|bk, row 128 = bv|16g.
    """
    fa = np.asarray(inputs["fa"], dtype=np.float32).reshape(B * C, N)
    fb = np.asarray(inputs["fb"], dtype=np.float32).reshape(B * C, N)
    Wq = np.asarray(inputs["Wq"], dtype=np.float32)
    Wk = np.asarray(inputs["Wk"], dtype=np.float32)
    Wv = np.asarray(inputs["Wv"], dtype=np.float32)
    bq = np.asarray(inputs["bq"], dtype=np.float32).reshape(CQ)
    bk = np.asarray(inputs["bk"], dtype=np.float32).reshape(CQ)
    bv = np.asarray(inputs["bv"], dtype=np.float32).reshape(C)
    gamma = float(np.asarray(inputs["gamma"]))

    wpack1 = np.ascontiguousarray(np.concatenate(
        [Wq.T, Wk.T, Wv.T * (1.0 / FA_SCALE)], axis=1).astype(NP_BF16))
    pk_u8 = np.empty((B * C, 3 * N + 2 * (2 * CQ + C)), np.uint8)
    pk_u8[:, 0:N] = np.ascontiguousarray(_quant_fa(fa)).view(np.uint8)
    pk_u8[:, N:3 * N] = np.ascontiguousarray(fb.astype(NP_BF16)).view(np.uint8)
    pk_u8[:, 3 * N:] = np.tile(wpack1.view(np.uint8), (B, 1))

    bqk1 = np.concatenate([bq, bk]).reshape(2 * CQ, 1).astype(np.float32)
    brow1 = np.concatenate(
        [bv, [Y_SCALE * gamma]]).reshape(1, C + 1).astype(np.float32)

    arrs = {
        "pk": pk_u8.view(NP_BF16),
        "bqk": np.tile(bqk1, (B, 1)),
        "brow": np.tile(brow1, (B, 1)),
    }
    return arrs, fa


def _dispatch_overlapped(inputs):
    """Convert + stage packed inputs, run, fetch y."""
    import jax
    fn, in_names, out_names = _get_dispatch()
    sh = _get_sharding()
    arrs, fa = _prep_inputs(inputs)
    dev = {n: jax.device_put(arrs[n], sh) for n in arrs}
    outs = fn(*[dev[n] for n in in_names])
    return np.asarray(outs[out_names.index("out")]), fa


def _postprocess(y_wire, fa_f32):
    """Wire output [B*C, N] e3m4 -> full relu(y/16 + fa) [B, C, H, W] f32."""
    y = np.asarray(y_wire).astype(np.float32)
    out = y * (1.0 / Y_SCALE)
    out += fa_f32
    np.maximum(out, 0.0, out=out)
    return out.reshape(B, C, HW, HW)


def kernel(**inputs):
    try:
        y, fa_f32 = _dispatch_overlapped(inputs)
    except Exception:
        # Fallback: per-core in_maps through the stock SPMD runner.
        from concourse.bass_utils import run_bass_kernel_spmd
        arrs, fa_f32 = _prep_inputs(inputs)
        nc = _get_nc()
        in_maps = []
        for b in range(B):
            in_maps.append({k: np.ascontiguousarray(
                v[b * (v.shape[0] // B):(b + 1) * (v.shape[0] // B)])
                for k, v in arrs.items()})
        res = run_bass_kernel_spmd(nc, in_maps, list(range(B))).results
        y = np.concatenate([res[b]["out"] for b in range(B)], axis=0)
    return _postprocess(y, fa_f32)
